# revision 10
# baseline (speedup 1.0000x reference)
"""Trainium2 Bass kernel for nn_CALayer (FFT-magnitude channel attention).

Math per (b, c) image X [256, 256] (real):
  F(p, q) = 2D DFT;  y[b,c] = mean over the centered (fftshifted) 100x100
  low-frequency crop of |F|;  s = sigmoid(w2 @ relu(w1 @ y + b1) + b2);
  out = x * s[:, :, None, None].

Implementation: DFT-as-matmul with Hermitian reduction. Since X is real,
|F(-p,-q)| = |F(p,q)|, so only p in 0..50 (51 rows) and q in -50..50
(101 cols) of the spectrum are computed, and the crop sum over
p,q in [-50, 49]^2 is recovered as two q-window sums:
  S = sum_{q in -50..49} sum_{p in 0..49} |F| + sum_{q in -49..50} sum_{p in 1..50} |F|.

v4 design notes (measured on HW):
  - All DMA (x in, out) rides the sync-engine HW queue in FIFO order:
    16 input tiles first, then the 16 scaled output tiles, whose triggers
    wait on per-group scale-done sems. HBM stays saturated: outputs
    stream the moment inputs drain. Consts ride the scalar queue (2 DMAs).
  - The final per-channel scale uses DVE TENSOR_SCALAR with an AP
    (per-partition f32) scalar - measured 3.3 elem/ns/partition vs 0.91
    for broadcast tensor_tensor: 8 ops of 512 fp16 cols per (b,g).
  - |F|^2 is computed in fp16 with a 1/64 input prescale on the ACT
    square (values fit fp16; 64/1e4 is folded into w1), halving the
    mag-add cost on DVE.
  - GpSimd is left idle on purpose: its DSP tensor ops run at <0.5
    elem/ns AND crush concurrent DVE throughput ~3-20x via SBUF port
    contention (measured).
  - PSUM->SBUF U copies split 2 DVE / 2 ACT per group; square+sqrt on
    ACT; adds+reduce+scale on DVE. Both engines land at ~60us busy.

Sharding: pure data parallel over batch: core i handles batches 2i, 2i+1.
"""

import os
import sys

for _p in (
    "/root/.axon_site",
    "/root/.axon_site/_ro/trn_rl_repo",
    "/root/.axon_site/_ro/pypackages",
    "/opt/trn_rl_repo",
):
    if os.path.isdir(_p) and _p not in sys.path:
        sys.path.append(_p)

import numpy as np

import concourse.bacc as bacc
import concourse.mybir as mybir
import concourse.tile as tile
from concourse.bass_utils import run_bass_kernel_spmd

N_CORES = 8
B, C, H, W = 16, 64, 256, 256
BPC = B // N_CORES  # batches per core
CROP = 50
NP_ = 51   # p = 0..50
NQ = 101   # q = -50..50
GS = 8     # channels per group
NG = C // GS
NU = 104  # 51 cos | pad | 51 (-sin) | pad
F32 = mybir.dt.float32
F16 = mybir.dt.float16
F8 = mybir.dt.float8e4
AF = mybir.ActivationFunctionType
ALU = mybir.AluOpType
MSCALE = 1.0 / 64.0  # |F| prescale so |F|^2 fits fp16


def _build_consts(w1, b1, w2, b2):
    h_idx = np.arange(H)
    ang_p = 2 * np.pi * np.outer(h_idx, np.arange(NP_)) / H
    wu = np.zeros((H, NU), np.float32)
    wu[:, 0:NP_] = np.cos(ang_p)
    wu[:, 52:52 + NP_] = -np.sin(ang_p)
    wu2 = wu.reshape(128, 2, NU)  # [p, k, n] with h = 2p + k
    ang_q = 2 * np.pi * np.outer(h_idx, np.arange(-CROP, CROP + 1)) / W
    cq = np.cos(ang_q).astype(np.float32)
    sq = np.sin(ang_q).astype(np.float32)
    wv = np.concatenate([cq, sq, -sq], axis=1)        # [256, 303]
    wv2 = wv.reshape(2, 128, 303).transpose(1, 0, 2)  # [p, k, :] w = 128k + p
    r1 = np.zeros((NQ, 2), np.float32)
    r1[0:100, 0] = 1.0  # q in -50..49
    r1[1:101, 1] = 1.0  # q in -49..50
    # fp16 blob [128, 210]: wu (208) | r1 (2)
    cb16 = np.zeros((128, 2 * NU + 2), np.float16)
    cb16[:, 0:208] = wu2.reshape(128, 208).astype(np.float16)
    cb16[0:NQ, 208:210] = r1.astype(np.float16)
    # fp8 wv blob [128, 2, 384]: cos@0, sin@128, -sin@256 (101 cols each)
    import ml_dtypes
    cb8 = np.zeros((128, 2, 384), np.float32)
    cb8[:, :, 0:101] = wv2[:, :, 0:101]
    cb8[:, :, 128:229] = wv2[:, :, 101:202]
    cb8[:, :, 256:357] = wv2[:, :, 202:303]
    cb8 = cb8.reshape(128, 768).astype(ml_dtypes.float8_e4m3)
    # one f32 blob [128, 133]: w1t | b1c | w2t | b2r
    # y on device = sum|F| * MSCALE; reference y = sum|F| / 1e4
    cb32 = np.zeros((128, 133), np.float32)
    cb32[0:C, 0:4] = w1.T.astype(np.float32) / (1e4 * MSCALE)
    cb32[0:4, 4] = b1.astype(np.float32)
    cb32[0:4, 5:69] = w2.T.astype(np.float32)
    cb32[0, 69:133] = b2.astype(np.float32)
    return {"cb16": cb16, "cb8": cb8, "cb32": cb32}


def _build_nc():
    nc = bacc.Bacc("TRN2", target_bir_lowering=False, debug=False)
    # x pre-transposed AND pre-converted to fp16 on host:
    # [b, p, c, k, w] with h = 2p + k
    x_d = nc.dram_tensor("x", [BPC, 128, C, 2, W], F16, kind="ExternalInput").ap()
    out_d = nc.dram_tensor("out", [BPC, 128, C, 2, W], F16, kind="ExternalOutput").ap()
    cb16_d = nc.dram_tensor("cb16", [128, 210], F16, kind="ExternalInput").ap()
    cb8_d = nc.dram_tensor("cb8", [128, 768], F8, kind="ExternalInput").ap()
    cb32_d = nc.dram_tensor("cb32", [128, 133], F32, kind="ExternalInput").ap()

    with tile.TileContext(nc) as tc:
        with (
            tc.tile_pool(name="consts", bufs=1) as cpool,
            tc.tile_pool(name="xr", bufs=2 * NG) as xpool,
            tc.tile_pool(name="us", bufs=3) as upool,
            tc.tile_pool(name="work", bufs=2) as wpool,
            tc.tile_pool(name="psA", bufs=2, space="PSUM") as pA,
            tc.tile_pool(name="psB", bufs=2, space="PSUM") as pB,
            tc.tile_pool(name="psS", bufs=1, space="PSUM") as pS,
        ):
            # ---- consts: two DMAs on the vector queue + on-device memsets
            cb16_sb = cpool.tile([128, 210], F16, name="cb16_sb")
            nc.scalar.dma_start(cb16_sb[:], cb16_d[:])
            cb8_sb = cpool.tile([128, 768], F8, name="cb8_sb")
            nc.scalar.dma_start(cb8_sb[:], cb8_d[:])
            cb32_sb = cpool.tile([128, 133], F32, name="cb32_sb")
            nc.scalar.dma_start(cb32_sb[:], cb32_d[:])
            wu_sb = cb16_sb[:, 0:208].rearrange("p (k n) -> p k n", k=2)
            wv_sb = cb8_sb.rearrange("p (k n) -> p k n", k=2)
            r1_sb = cb16_sb[0:NQ, 208:210]
            w1t_sb = cb32_sb[0:C, 0:4]
            b1c_sb = cb32_sb[0:4, 4:5]
            w2t_sb = cb32_sb[0:4, 5:69]
            b2r_sb = cb32_sb[0:1, 69:133]
            ones128_sb = cpool.tile([1, 128], F32, name="ones128_sb")
            nc.vector.memset(ones128_sb[:], 1.0)
            id1_sb = cpool.tile([1, 1], F32, name="id1_sb")
            nc.vector.memset(id1_sb[:], 1.0)
            zeros4_sb = cpool.tile([4, 1], F32, name="zeros4_sb")
            nc.vector.memset(zeros4_sb[:], 0.0)
            sigwarm = cpool.tile([1, 1], F32, name="sigwarm")
            nc.scalar.activation(sigwarm[:], id1_sb[:], AF.Sigmoid)

            # ---- all 16 input DMAs upfront on the sync HW queue
            xrs = {}
            for b in range(BPC):
                for g in range(NG):
                    xr = xpool.tile([128, GS, 2, W], F16, name="xr", tag="xr")
                    nc.sync.dma_start(xr[:], x_d[b, :, GS * g:GS * (g + 1)])
                    xrs[(b, g)] = xr

            ys = {}
            us = {}
            sbs = {}

            def emit_A(b, g):
                if g == 0:
                    ys[b] = wpool.tile([1, C], F32, name="y_sb", tag="y")
                xr = xrs[(b, g)]
                # step A: U^T = X^T @ Wu per channel (both w-chunks),
                # PSUM batched over pairs of channels
                u_sb = upool.tile([128, GS, 2, NU], F8, name="u_sb", tag="u")
                for jb in range(GS // 2):
                    psA = pA.tile([128, 2, 2, NU], F32, name="psA", tag="uA")
                    for jj in range(2):
                        j = 2 * jb + jj
                        for wk in range(2):
                            for kk in range(2):
                                nc.tensor.matmul(
                                    psA[:, jj, wk, :],
                                    xr[:, j, kk, 128 * wk:128 * (wk + 1)],
                                    wu_sb[:, kk, :],
                                    start=(kk == 0),
                                    stop=(kk == 1),
                                )
                    dst = u_sb[:, 2 * jb:2 * jb + 2]
                    n_dve = 1 if (b * NG + g) % 4 == 0 else 0
                    if jb < n_dve:
                        nc.vector.tensor_copy(dst, psA[:])
                    else:
                        nc.scalar.copy(dst, psA[:])
                us[(b, g)] = u_sb

            def emit_Bmagcrop(b, g):
                y_sb = ys[b]
                u_sb = us[(b, g)]
                # step B: F^T[q, (ch, p)] with complex arithmetic
                psB = pB.tile([102, 1024], F32, name="psB", tag="fB")
                fr = psB[:, 0:416]
                fi = psB[:, 512:928]
                # fp8 DoubleRow: one matmul contracts both w-halves (k dim)
                ur = u_sb[:, :, :, 0:52].rearrange("p c k n -> p k c n")
                ui = u_sb[:, :, :, 52:104].rearrange("p c k n -> p k c n")
                ck = wv_sb[:, :, 0:102]
                sk = wv_sb[:, :, 128:230]
                snk = wv_sb[:, :, 256:358]
                DR = mybir.MatmulPerfMode.DoubleRow
                nc.tensor.matmul(fr, ck, ur, start=True, stop=False, perf_mode=DR)
                nc.tensor.matmul(fr, sk, ui, start=False, stop=True, perf_mode=DR)
                nc.tensor.matmul(fi, ck, ui, start=True, stop=False, perf_mode=DR)
                nc.tensor.matmul(fi, snk, ur, start=False, stop=True, perf_mode=DR)

                # |F|*MSCALE in fp16: one prescaled ACT square over the
                # (fr, fi) pair, fp16 add on DVE, sqrt on ACT
                m2 = wpool.tile([NQ, 2, 416], F16, name="m2", tag="m2")
                pair = psB.rearrange("p (a x) -> p a x", a=2)[0:NQ, :, 0:416]
                nc.scalar.activation(m2[:], pair, AF.Square, scale=MSCALE)
                nc.vector.tensor_add(m2[:, 0], m2[:, 0], m2[:, 1])
                mag = wpool.tile([NQ, 416], F16, name="mag", tag="mag")
                nc.scalar.sqrt(mag[:], m2[:, 0])

                # crop sum: both q-window matmuls accumulate in PSUM,
                # then one windowed free-dim reduce -> y row slice
                mag3 = mag.rearrange("p (c x) -> p c x", c=GS)
                g2 = pS.tile([1, 400], F32, name="g2", tag="G")
                nc.tensor.matmul(
                    g2[:], r1_sb[:, 0:1], mag3[:, :, 0:50], start=True, stop=False
                )
                nc.tensor.matmul(
                    g2[:], r1_sb[:, 1:2], mag3[:, :, 1:51], start=False, stop=True
                )
                ga = g2.rearrange("p (c x) -> p c x", c=GS)
                nc.vector.reduce_sum(
                    y_sb[0:1, GS * g:GS * (g + 1)], ga, axis=mybir.AxisListType.X
                )

            def emit_se(b):
                # SE block (mean divisor 1e4 and MSCALE folded into w1t)
                y_sb = ys[b]
                yT_ps = pS.tile([C, 1], F32, name="yT_ps", tag="se")
                nc.tensor.transpose(yT_ps[:], y_sb[:], id1_sb[:])
                y_col = wpool.tile([C, 1], F32, name="y_col", tag="se2")
                nc.vector.tensor_copy(y_col[:], yT_ps[:])
                h_ps = pS.tile([4, 1], F32, name="h_ps", tag="se")
                nc.tensor.matmul(h_ps[:], w1t_sb, y_col[:], start=True, stop=True)
                h_sb = wpool.tile([4, 1], F32, name="h_sb", tag="se3")
                # relu(h + b1) without an ACT Relu table:
                nc.vector.scalar_tensor_tensor(
                    h_sb[:], h_ps[:], b1c_sb, zeros4_sb[:], ALU.add, ALU.max
                )
                sarg_ps = pS.tile([1, C], F32, name="sarg_ps", tag="se")
                nc.tensor.matmul(sarg_ps[:], h_sb[:], w2t_sb, start=True, stop=True)
                sarg_sb = wpool.tile([1, C], F32, name="sarg_sb", tag="se4")
                nc.vector.tensor_add(sarg_sb[:], sarg_ps[:], b2r_sb)
                s_row = wpool.tile([1, C], F32, name="s_row", tag="se5")
                nc.scalar.activation(s_row[:], sarg_sb[:], AF.Sigmoid)
                sb_ps = pS.tile([128, C], F32, name="sb_ps", tag="se")
                nc.tensor.matmul(
                    sb_ps[:], ones128_sb[:], s_row[:], start=True, stop=True
                )
                s_b = wpool.tile([128, C], F32, name="s_b", tag="se6")
                nc.vector.tensor_copy(s_b[:], sb_ps[:])
                sbs[b] = s_b

            def emit_scale(b, g):
                # in-place fp16 per-channel scale: DVE TENSOR_SCALAR with
                # per-partition f32 AP scalar (fast path, ~340ns/channel)
                xr = xrs[(b, g)]
                s_b = sbs[b]
                for j in range(GS):
                    c = GS * g + j
                    nc.vector.tensor_scalar_mul(
                        xr[:, j], xr[:, j], s_b[:, c:c + 1]
                    )

            # software pipeline: A(u+1) before B/mag/crop(u); SE(0) as soon
            # as batch-0 crops are done; batch-0 scale interleaved with
            # batch-1's FFT stream.
            units = [(b, g) for b in range(BPC) for g in range(NG)]
            for u, (b, g) in enumerate(units):
                emit_A(b, g)
                if u >= 1:
                    emit_Bmagcrop(*units[u - 1])
                if u == NG:
                    emit_se(0)
                if NG + 1 <= u:
                    emit_scale(0, u - NG - 1)
            emit_Bmagcrop(*units[-1])
            emit_scale(0, NG - 1)
            emit_se(1)
            for g in range(NG):
                emit_scale(1, g)

            # output DMAs: sync HW queue, FIFO behind the 16 inputs;
            # each trigger waits only on its group's scale completion.
            for b in range(BPC):
                for g in range(NG):
                    nc.sync.dma_start(
                        out_d[b, :, GS * g:GS * (g + 1)], xrs[(b, g)][:]
                    )

    nc.compile()
    return nc


_NC = None


def _get_nc():
    global _NC
    if _NC is None:
        _NC = _build_nc()
    return _NC


def _execute(inputs, trace=False):
    x = np.asarray(inputs["x"], dtype=np.float32)
    consts = _build_consts(
        np.asarray(inputs["w1"]), np.asarray(inputs["b1"]),
        np.asarray(inputs["w2"]), np.asarray(inputs["b2"]),
    )
    in_maps = []
    for i in range(N_CORES):
        xs = x[BPC * i:BPC * (i + 1)]
        # [b, c, (p k), w] -> [b, p, c, k, w]  (h = 2p + k), fp16
        xs = np.ascontiguousarray(
            xs.reshape(BPC, C, 128, 2, W).transpose(0, 2, 1, 3, 4),
            dtype=np.float16,
        )
        m = {"x": xs}
        m.update(consts)
        in_maps.append(m)
    nc = _get_nc()
    res = run_bass_kernel_spmd(nc, in_maps, core_ids=list(range(N_CORES)), trace=trace)
    outs = []
    for i in range(N_CORES):
        o = res.results[i]["out"]  # [b, p, c, k, w] fp16
        o = o.transpose(0, 2, 1, 3, 4).reshape(BPC, C, H, W).astype(np.float32)
        outs.append(o)
    out = np.concatenate(outs, axis=0)
    return out, res


def kernel(x, w1, b1, w2, b2):
    out, _ = _execute({"x": x, "w1": w1, "b1": b1, "w2": w2, "b2": b2}, trace=False)
    return out


# revision 11
# speedup vs baseline: 1.0192x; 1.0192x over previous
"""Trainium2 Bass kernel for nn_CALayer (FFT-magnitude channel attention).

Math per (b, c) image X [256, 256] (real):
  F(p, q) = 2D DFT;  y[b,c] = mean over the centered (fftshifted) 100x100
  low-frequency crop of |F|;  s = sigmoid(w2 @ relu(w1 @ y + b1) + b2);
  out = x * s[:, :, None, None].

Implementation: DFT-as-matmul with Hermitian reduction. Since X is real,
|F(-p,-q)| = |F(p,q)|, so only p in 0..50 (51 rows) and q in -50..50
(101 cols) of the spectrum are computed, and the crop sum over
p,q in [-50, 49]^2 is recovered as two q-window sums:
  S = sum_{q in -50..49} sum_{p in 0..49} |F| + sum_{q in -49..50} sum_{p in 1..50} |F|.

v4 design notes (measured on HW):
  - All DMA (x in, out) rides the sync-engine HW queue in FIFO order:
    16 input tiles first, then the 16 scaled output tiles, whose triggers
    wait on per-group scale-done sems. HBM stays saturated: outputs
    stream the moment inputs drain. Consts ride the scalar queue (2 DMAs).
  - The final per-channel scale uses DVE TENSOR_SCALAR with an AP
    (per-partition f32) scalar - measured 3.3 elem/ns/partition vs 0.91
    for broadcast tensor_tensor: 8 ops of 512 fp16 cols per (b,g).
  - |F|^2 is computed in fp16 with a 1/64 input prescale on the ACT
    square (values fit fp16; 64/1e4 is folded into w1), halving the
    mag-add cost on DVE.
  - GpSimd is left idle on purpose: its DSP tensor ops run at <0.5
    elem/ns AND crush concurrent DVE throughput ~3-20x via SBUF port
    contention (measured).
  - PSUM->SBUF U copies split 2 DVE / 2 ACT per group; square+sqrt on
    ACT; adds+reduce+scale on DVE. Both engines land at ~60us busy.

Sharding: pure data parallel over batch: core i handles batches 2i, 2i+1.
"""

import os
import sys

for _p in (
    "/root/.axon_site",
    "/root/.axon_site/_ro/trn_rl_repo",
    "/root/.axon_site/_ro/pypackages",
    "/opt/trn_rl_repo",
):
    if os.path.isdir(_p) and _p not in sys.path:
        sys.path.append(_p)

import numpy as np

import concourse.bacc as bacc
import concourse.mybir as mybir
import concourse.tile as tile
from concourse.bass_utils import run_bass_kernel_spmd

N_CORES = 8
B, C, H, W = 16, 64, 256, 256
BPC = B // N_CORES  # batches per core
CROP = 50
NP_ = 51   # p = 0..50
NQ = 101   # q = -50..50
GS = 8     # channels per group
NG = C // GS
NU = 104  # 51 cos | pad | 51 (-sin) | pad
F32 = mybir.dt.float32
F16 = mybir.dt.float16
F8 = mybir.dt.float8e4
AF = mybir.ActivationFunctionType
ALU = mybir.AluOpType
MSCALE = 1.0 / 64.0  # |F| prescale so |F|^2 fits fp16


def _build_consts(w1, b1, w2, b2):
    h_idx = np.arange(H)
    ang_p = 2 * np.pi * np.outer(h_idx, np.arange(NP_)) / H
    wu = np.zeros((H, NU), np.float32)
    wu[:, 0:NP_] = np.cos(ang_p)
    wu[:, 52:52 + NP_] = -np.sin(ang_p)
    wu2 = wu.reshape(128, 2, NU)  # [p, k, n] with h = 2p + k
    ang_q = 2 * np.pi * np.outer(h_idx, np.arange(-CROP, CROP + 1)) / W
    cq = np.cos(ang_q).astype(np.float32)
    sq = np.sin(ang_q).astype(np.float32)
    wv = np.concatenate([cq, sq, -sq], axis=1)        # [256, 303]
    wv2 = wv.reshape(2, 128, 303).transpose(1, 0, 2)  # [p, k, :] w = 128k + p
    r1 = np.zeros((NQ, 2), np.float32)
    r1[0:100, 0] = 1.0  # q in -50..49
    r1[1:101, 1] = 1.0  # q in -49..50
    # fp16 blob [128, 210]: wu (208) | r1 (2)
    cb16 = np.zeros((128, 2 * NU + 2), np.float16)
    cb16[:, 0:208] = wu2.reshape(128, 208).astype(np.float16)
    cb16[0:NQ, 208:210] = r1.astype(np.float16)
    # fp8 wv blob [128, 2, 384]: cos@0, sin@128, -sin@256 (101 cols each)
    import ml_dtypes
    cb8 = np.zeros((128, 2, 384), np.float32)
    cb8[:, :, 0:101] = wv2[:, :, 0:101]
    cb8[:, :, 128:229] = wv2[:, :, 101:202]
    cb8[:, :, 256:357] = wv2[:, :, 202:303]
    cb8 = cb8.reshape(128, 768).astype(ml_dtypes.float8_e4m3)
    # one f32 blob [128, 133]: w1t | b1c | w2t | b2r
    # y on device = sum|F| * MSCALE; reference y = sum|F| / 1e4
    cb32 = np.zeros((128, 133), np.float32)
    cb32[0:C, 0:4] = w1.T.astype(np.float32) / (1e4 * MSCALE)
    cb32[0:4, 4] = b1.astype(np.float32)
    cb32[0:4, 5:69] = w2.T.astype(np.float32)
    cb32[0, 69:133] = b2.astype(np.float32)
    return {"cb16": cb16, "cb8": cb8, "cb32": cb32}


def _build_nc():
    nc = bacc.Bacc("TRN2", target_bir_lowering=False, debug=False)
    # x pre-transposed AND pre-converted to fp16 on host:
    # [b, p, c, k, w] with h = 2p + k
    x_d = nc.dram_tensor("x", [BPC, 128, C, 2, W], F16, kind="ExternalInput").ap()
    out_d = nc.dram_tensor("out", [BPC, 128, C, 2, W], F16, kind="ExternalOutput").ap()
    cb16_d = nc.dram_tensor("cb16", [128, 210], F16, kind="ExternalInput").ap()
    cb8_d = nc.dram_tensor("cb8", [128, 768], F8, kind="ExternalInput").ap()
    cb32_d = nc.dram_tensor("cb32", [128, 133], F32, kind="ExternalInput").ap()

    with tile.TileContext(nc) as tc:
        with (
            tc.tile_pool(name="consts", bufs=1) as cpool,
            tc.tile_pool(name="xr", bufs=2 * NG) as xpool,
            tc.tile_pool(name="us", bufs=3) as upool,
            tc.tile_pool(name="work", bufs=2) as wpool,
            tc.tile_pool(name="psA", bufs=2, space="PSUM") as pA,
            tc.tile_pool(name="psB", bufs=2, space="PSUM") as pB,
            tc.tile_pool(name="psS", bufs=1, space="PSUM") as pS,
        ):
            # ---- consts: two DMAs on the vector queue + on-device memsets
            cb16_sb = cpool.tile([128, 210], F16, name="cb16_sb")
            nc.scalar.dma_start(cb16_sb[:], cb16_d[:])
            cb8_sb = cpool.tile([128, 768], F8, name="cb8_sb")
            nc.scalar.dma_start(cb8_sb[:], cb8_d[:])
            cb32_sb = cpool.tile([128, 133], F32, name="cb32_sb")
            nc.scalar.dma_start(cb32_sb[:], cb32_d[:])
            wu_sb = cb16_sb[:, 0:208].rearrange("p (k n) -> p k n", k=2)
            wv_sb = cb8_sb.rearrange("p (k n) -> p k n", k=2)
            r1_sb = cb16_sb[0:NQ, 208:210]
            w1t_sb = cb32_sb[0:C, 0:4]
            b1c_sb = cb32_sb[0:4, 4:5]
            w2t_sb = cb32_sb[0:4, 5:69]
            b2r_sb = cb32_sb[0:1, 69:133]
            ones128_sb = cpool.tile([1, 128], F32, name="ones128_sb")
            nc.vector.memset(ones128_sb[:], 1.0)
            id1_sb = cpool.tile([1, 1], F32, name="id1_sb")
            nc.vector.memset(id1_sb[:], 1.0)
            zeros4_sb = cpool.tile([4, 1], F32, name="zeros4_sb")
            nc.vector.memset(zeros4_sb[:], 0.0)
            sigwarm = cpool.tile([1, 1], F32, name="sigwarm")
            nc.scalar.activation(sigwarm[:], id1_sb[:], AF.Sigmoid)

            # ---- all 16 input DMAs upfront on the sync HW queue
            xrs = {}
            for b in range(BPC):
                for g in range(NG):
                    xr = xpool.tile([128, GS, 2, W], F16, name="xr", tag="xr")
                    nc.sync.dma_start(xr[:], x_d[b, :, GS * g:GS * (g + 1)])
                    xrs[(b, g)] = xr

            ys = {}
            us = {}
            sbs = {}

            def emit_A(b, g):
                if g == 0:
                    ys[b] = wpool.tile([1, C], F32, name="y_sb", tag="y")
                xr = xrs[(b, g)]
                # step A: U^T = X^T @ Wu per channel (both w-chunks),
                # PSUM batched over pairs of channels
                u_sb = upool.tile([128, GS, 2, NU], F8, name="u_sb", tag="u")
                for jb in range(GS // 2):
                    psA = pA.tile([128, 2, 2, NU], F32, name="psA", tag="uA")
                    for jj in range(2):
                        j = 2 * jb + jj
                        for wk in range(2):
                            for kk in range(2):
                                nc.tensor.matmul(
                                    psA[:, jj, wk, :],
                                    xr[:, j, kk, 128 * wk:128 * (wk + 1)],
                                    wu_sb[:, kk, :],
                                    start=(kk == 0),
                                    stop=(kk == 1),
                                )
                    dst = u_sb[:, 2 * jb:2 * jb + 2]
                    if jb < 2:
                        nc.vector.tensor_copy(dst, psA[:])
                    else:
                        nc.scalar.copy(dst, psA[:])
                us[(b, g)] = u_sb

            def emit_Bmagcrop(b, g):
                y_sb = ys[b]
                u_sb = us[(b, g)]
                # step B: F^T[q, (ch, p)] with complex arithmetic
                psB = pB.tile([102, 1024], F32, name="psB", tag="fB")
                fr = psB[:, 0:416]
                fi = psB[:, 512:928]
                # fp8 DoubleRow: one matmul contracts both w-halves (k dim)
                ur = u_sb[:, :, :, 0:52].rearrange("p c k n -> p k c n")
                ui = u_sb[:, :, :, 52:104].rearrange("p c k n -> p k c n")
                ck = wv_sb[:, :, 0:102]
                sk = wv_sb[:, :, 128:230]
                snk = wv_sb[:, :, 256:358]
                DR = mybir.MatmulPerfMode.DoubleRow
                nc.tensor.matmul(fr, ck, ur, start=True, stop=False, perf_mode=DR)
                nc.tensor.matmul(fr, sk, ui, start=False, stop=True, perf_mode=DR)
                nc.tensor.matmul(fi, ck, ui, start=True, stop=False, perf_mode=DR)
                nc.tensor.matmul(fi, snk, ur, start=False, stop=True, perf_mode=DR)

                # |F|*MSCALE in fp16: one prescaled ACT square over the
                # (fr, fi) pair, fp16 add on DVE, sqrt on ACT
                m2 = wpool.tile([NQ, 2, 416], F16, name="m2", tag="m2")
                pair = psB.rearrange("p (a x) -> p a x", a=2)[0:NQ, :, 0:416]
                nc.scalar.activation(m2[:], pair, AF.Square, scale=MSCALE)
                nc.vector.tensor_add(m2[:, 0], m2[:, 0], m2[:, 1])
                mag = wpool.tile([NQ, 416], F16, name="mag", tag="mag")
                nc.scalar.sqrt(mag[:], m2[:, 0])

                # crop sum: both q-window matmuls accumulate in PSUM,
                # then one windowed free-dim reduce -> y row slice
                mag3 = mag.rearrange("p (c x) -> p c x", c=GS)
                g2 = pS.tile([1, 400], F32, name="g2", tag="G")
                nc.tensor.matmul(
                    g2[:], r1_sb[:, 0:1], mag3[:, :, 0:50], start=True, stop=False
                )
                nc.tensor.matmul(
                    g2[:], r1_sb[:, 1:2], mag3[:, :, 1:51], start=False, stop=True
                )
                ga = g2.rearrange("p (c x) -> p c x", c=GS)
                nc.vector.reduce_sum(
                    y_sb[0:1, GS * g:GS * (g + 1)], ga, axis=mybir.AxisListType.X
                )

            def emit_se(b):
                # SE block (mean divisor 1e4 and MSCALE folded into w1t)
                y_sb = ys[b]
                yT_ps = pS.tile([C, 1], F32, name="yT_ps", tag="se")
                nc.tensor.transpose(yT_ps[:], y_sb[:], id1_sb[:])
                y_col = wpool.tile([C, 1], F32, name="y_col", tag="se2")
                nc.vector.tensor_copy(y_col[:], yT_ps[:])
                h_ps = pS.tile([4, 1], F32, name="h_ps", tag="se")
                nc.tensor.matmul(h_ps[:], w1t_sb, y_col[:], start=True, stop=True)
                h_sb = wpool.tile([4, 1], F32, name="h_sb", tag="se3")
                # relu(h + b1) without an ACT Relu table:
                nc.vector.scalar_tensor_tensor(
                    h_sb[:], h_ps[:], b1c_sb, zeros4_sb[:], ALU.add, ALU.max
                )
                sarg_ps = pS.tile([1, C], F32, name="sarg_ps", tag="se")
                nc.tensor.matmul(sarg_ps[:], h_sb[:], w2t_sb, start=True, stop=True)
                sarg_sb = wpool.tile([1, C], F32, name="sarg_sb", tag="se4")
                nc.vector.tensor_add(sarg_sb[:], sarg_ps[:], b2r_sb)
                s_row = wpool.tile([1, C], F32, name="s_row", tag="se5")
                nc.scalar.activation(s_row[:], sarg_sb[:], AF.Sigmoid)
                sb_ps = pS.tile([128, C], F32, name="sb_ps", tag="se")
                nc.tensor.matmul(
                    sb_ps[:], ones128_sb[:], s_row[:], start=True, stop=True
                )
                s_b = wpool.tile([128, C], F32, name="s_b", tag="se6")
                nc.vector.tensor_copy(s_b[:], sb_ps[:])
                sbs[b] = s_b

            def emit_scale(b, g):
                # in-place fp16 per-channel scale: DVE TENSOR_SCALAR with
                # per-partition f32 AP scalar (fast path, ~340ns/channel)
                xr = xrs[(b, g)]
                s_b = sbs[b]
                for j in range(GS):
                    c = GS * g + j
                    nc.vector.tensor_scalar_mul(
                        xr[:, j], xr[:, j], s_b[:, c:c + 1]
                    )

            # software pipeline: A(u+1) before B/mag/crop(u); SE(0) as soon
            # as batch-0 crops are done; batch-0 scale interleaved with
            # batch-1's FFT stream.
            units = [(b, g) for b in range(BPC) for g in range(NG)]
            for u, (b, g) in enumerate(units):
                emit_A(b, g)
                if u >= 1:
                    emit_Bmagcrop(*units[u - 1])
                if u == NG:
                    emit_se(0)
                if NG + 1 <= u:
                    emit_scale(0, u - NG - 1)
            emit_Bmagcrop(*units[-1])
            emit_scale(0, NG - 1)
            emit_se(1)
            for g in range(NG):
                emit_scale(1, g)

            # output DMAs: sync HW queue, FIFO behind the 16 inputs;
            # each trigger waits only on its group's scale completion.
            for b in range(BPC):
                for g in range(NG):
                    nc.sync.dma_start(
                        out_d[b, :, GS * g:GS * (g + 1)], xrs[(b, g)][:]
                    )

    nc.compile()
    return nc


_NC = None


def _get_nc():
    global _NC
    if _NC is None:
        _NC = _build_nc()
    return _NC


def _execute(inputs, trace=False):
    x = np.asarray(inputs["x"], dtype=np.float32)
    consts = _build_consts(
        np.asarray(inputs["w1"]), np.asarray(inputs["b1"]),
        np.asarray(inputs["w2"]), np.asarray(inputs["b2"]),
    )
    in_maps = []
    for i in range(N_CORES):
        xs = x[BPC * i:BPC * (i + 1)]
        # [b, c, (p k), w] -> [b, p, c, k, w]  (h = 2p + k), fp16
        xs = np.ascontiguousarray(
            xs.reshape(BPC, C, 128, 2, W).transpose(0, 2, 1, 3, 4),
            dtype=np.float16,
        )
        m = {"x": xs}
        m.update(consts)
        in_maps.append(m)
    nc = _get_nc()
    res = run_bass_kernel_spmd(nc, in_maps, core_ids=list(range(N_CORES)), trace=trace)
    outs = []
    for i in range(N_CORES):
        o = res.results[i]["out"]  # [b, p, c, k, w] fp16
        o = o.transpose(0, 2, 1, 3, 4).reshape(BPC, C, H, W).astype(np.float32)
        outs.append(o)
    out = np.concatenate(outs, axis=0)
    return out, res


def kernel(x, w1, b1, w2, b2):
    out, _ = _execute({"x": x, "w1": w1, "b1": b1, "w2": w2, "b2": b2}, trace=False)
    return out


# revision 12
# speedup vs baseline: 1.1538x; 1.1321x over previous
"""Trainium2 Bass kernel for nn_CALayer (FFT-magnitude channel attention).

Math per (b, c) image X [256, 256] (real):
  F(p, q) = 2D DFT;  y[b,c] = mean over the centered (fftshifted) 100x100
  low-frequency crop of |F|;  s = sigmoid(w2 @ relu(w1 @ y + b1) + b2);
  out = x * s[:, :, None, None].

Implementation: DFT-as-matmul with Hermitian reduction. Since X is real,
|F(-p,-q)| = |F(p,q)|, so only p in 0..50 (51 rows) and q in -50..50
(101 cols) of the spectrum are computed, and the crop sum over
p,q in [-50, 49]^2 is recovered as two q-window sums:
  S = sum_{q in -50..49} sum_{p in 0..49} |F| + sum_{q in -49..50} sum_{p in 1..50} |F|.

v4 design notes (measured on HW):
  - All DMA (x in, out) rides the sync-engine HW queue in FIFO order:
    16 input tiles first, then the 16 scaled output tiles, whose triggers
    wait on per-group scale-done sems. HBM stays saturated: outputs
    stream the moment inputs drain. Consts ride the scalar queue (2 DMAs).
  - The final per-channel scale uses DVE TENSOR_SCALAR with an AP
    (per-partition f32) scalar - measured 3.3 elem/ns/partition vs 0.91
    for broadcast tensor_tensor: 8 ops of 512 fp16 cols per (b,g).
  - |F|^2 is computed in fp16 with a 1/64 input prescale on the ACT
    square (values fit fp16; 64/1e4 is folded into w1), halving the
    mag-add cost on DVE.
  - GpSimd is left idle on purpose: its DSP tensor ops run at <0.5
    elem/ns AND crush concurrent DVE throughput ~3-20x via SBUF port
    contention (measured).
  - PSUM->SBUF U copies split 2 DVE / 2 ACT per group; square+sqrt on
    ACT; adds+reduce+scale on DVE. Both engines land at ~60us busy.

Sharding: pure data parallel over batch: core i handles batches 2i, 2i+1.
"""

import os
import sys

for _p in (
    "/root/.axon_site",
    "/root/.axon_site/_ro/trn_rl_repo",
    "/root/.axon_site/_ro/pypackages",
    "/opt/trn_rl_repo",
):
    if os.path.isdir(_p) and _p not in sys.path:
        sys.path.append(_p)

import numpy as np

import concourse.bacc as bacc
import concourse.mybir as mybir
import concourse.tile as tile
from concourse.bass_utils import run_bass_kernel_spmd

N_CORES = 8
B, C, H, W = 16, 64, 256, 256
BPC = B // N_CORES  # batches per core
CROP = 50
NP_ = 51   # p = 0..50
NQ = 101   # q = -50..50
GS = 8     # channels per group
NG = C // GS
NU = 104  # 51 cos | pad | 51 (-sin) | pad
F32 = mybir.dt.float32
F16 = mybir.dt.float16
F8 = mybir.dt.float8e4
AF = mybir.ActivationFunctionType
ALU = mybir.AluOpType
MSCALE = 1.0 / 64.0  # |F| prescale so |F|^2 fits fp16


def _build_consts(w1, b1, w2, b2):
    h_idx = np.arange(H)
    ang_p = 2 * np.pi * np.outer(h_idx, np.arange(NP_)) / H
    wu = np.zeros((H, NU), np.float32)
    wu[:, 0:NP_] = np.cos(ang_p)
    wu[:, 52:52 + NP_] = -np.sin(ang_p)
    wu2 = wu.reshape(128, 2, NU)  # [p, k, n] with h = 2p + k
    ang_q = 2 * np.pi * np.outer(h_idx, np.arange(-CROP, CROP + 1)) / W
    cq = np.cos(ang_q).astype(np.float32)
    sq = np.sin(ang_q).astype(np.float32)
    wv = np.concatenate([cq, sq, -sq], axis=1)        # [256, 303]
    wv2 = wv.reshape(2, 128, 303).transpose(1, 0, 2)  # [p, k, :] w = 128k + p
    r1 = np.zeros((NQ, 2), np.float32)
    r1[0:100, 0] = 1.0  # q in -50..49
    r1[1:101, 1] = 1.0  # q in -49..50
    # fp16 blob [128, 816]: wu (208) | wv (606) | r1 (2)
    cb16 = np.zeros((128, 2 * NU + 606 + 2), np.float16)
    cb16[:, 0:208] = wu2.reshape(128, 208).astype(np.float16)
    cb16[:, 208:814] = np.ascontiguousarray(wv2).reshape(128, 606).astype(np.float16)
    cb16[0:NQ, 814:816] = r1.astype(np.float16)
    # one f32 blob [128, 133]: w1t | b1c | w2t | b2r
    # y on device = sum|F| * MSCALE; reference y = sum|F| / 1e4
    cb32 = np.zeros((128, 133), np.float32)
    cb32[0:C, 0:4] = w1.T.astype(np.float32) / (1e4 * MSCALE)
    cb32[0:4, 4] = b1.astype(np.float32)
    cb32[0:4, 5:69] = w2.T.astype(np.float32)
    cb32[0, 69:133] = b2.astype(np.float32)
    return {"cb16": cb16, "cb32": cb32}


def _build_nc():
    nc = bacc.Bacc("TRN2", target_bir_lowering=False, debug=False)
    # x pre-transposed AND pre-converted to fp16 on host:
    # [b, p, c, k, w] with h = 2p + k
    x_d = nc.dram_tensor("x", [BPC, 128, C, 2, W], F16, kind="ExternalInput").ap()
    out_d = nc.dram_tensor("out", [BPC, 128, C, 2, W], F16, kind="ExternalOutput").ap()
    cb16_d = nc.dram_tensor("cb16", [128, 816], F16, kind="ExternalInput").ap()
    cb32_d = nc.dram_tensor("cb32", [128, 133], F32, kind="ExternalInput").ap()

    with tile.TileContext(nc) as tc:
        with (
            tc.tile_pool(name="consts", bufs=1) as cpool,
            tc.tile_pool(name="xr", bufs=2 * NG) as xpool,
            tc.tile_pool(name="us", bufs=3) as upool,
            tc.tile_pool(name="work", bufs=2) as wpool,
            tc.tile_pool(name="psA", bufs=2, space="PSUM") as pA,
            tc.tile_pool(name="psB", bufs=2, space="PSUM") as pB,
            tc.tile_pool(name="psS", bufs=1, space="PSUM") as pS,
        ):
            # ---- consts: two DMAs on the vector queue + on-device memsets
            cb16_sb = cpool.tile([128, 816], F16, name="cb16_sb")
            nc.scalar.dma_start(cb16_sb[:], cb16_d[:])
            cb32_sb = cpool.tile([128, 133], F32, name="cb32_sb")
            nc.scalar.dma_start(cb32_sb[:], cb32_d[:])
            wu_sb = cb16_sb[:, 0:208].rearrange("p (k n) -> p k n", k=2)
            wv_sb = cb16_sb[:, 208:814].rearrange("p (k n) -> p k n", k=2)
            r1_sb = cb16_sb[0:NQ, 814:816]
            w1t_sb = cb32_sb[0:C, 0:4]
            b1c_sb = cb32_sb[0:4, 4:5]
            w2t_sb = cb32_sb[0:4, 5:69]
            b2r_sb = cb32_sb[0:1, 69:133]
            ones128_sb = cpool.tile([1, 128], F32, name="ones128_sb")
            nc.vector.memset(ones128_sb[:], 1.0)
            id1_sb = cpool.tile([1, 1], F32, name="id1_sb")
            nc.vector.memset(id1_sb[:], 1.0)
            zeros4_sb = cpool.tile([4, 1], F32, name="zeros4_sb")
            nc.vector.memset(zeros4_sb[:], 0.0)
            sigwarm = cpool.tile([1, 1], F32, name="sigwarm")
            nc.scalar.activation(sigwarm[:], id1_sb[:], AF.Sigmoid)

            # ---- all 16 input DMAs upfront on the sync HW queue
            xrs = {}
            for b in range(BPC):
                for g in range(NG):
                    xr = xpool.tile([128, GS, 2, W], F16, name="xr", tag="xr")
                    nc.sync.dma_start(xr[:], x_d[b, :, GS * g:GS * (g + 1)])
                    xrs[(b, g)] = xr

            ys = {}
            us = {}
            sbs = {}

            def emit_A(b, g):
                if g == 0:
                    ys[b] = wpool.tile([1, C], F32, name="y_sb", tag="y")
                xr = xrs[(b, g)]
                # step A: U^T = X^T @ Wu per channel (both w-chunks),
                # PSUM batched over pairs of channels
                u_sb = upool.tile([128, GS, 2, NU], F16, name="u_sb", tag="u")
                for jb in range(GS // 2):
                    psA = pA.tile([128, 2, 2, NU], F32, name="psA", tag="uA")
                    for jj in range(2):
                        j = 2 * jb + jj
                        for wk in range(2):
                            for kk in range(2):
                                nc.tensor.matmul(
                                    psA[:, jj, wk, :],
                                    xr[:, j, kk, 128 * wk:128 * (wk + 1)],
                                    wu_sb[:, kk, :],
                                    start=(kk == 0),
                                    stop=(kk == 1),
                                )
                    dst = u_sb[:, 2 * jb:2 * jb + 2]
                    if jb < 2:
                        nc.vector.tensor_copy(dst, psA[:])
                    else:
                        nc.scalar.copy(dst, psA[:])
                us[(b, g)] = u_sb

            def emit_Bmagcrop(b, g):
                y_sb = ys[b]
                u_sb = us[(b, g)]
                # step B: F^T[q, (ch, p)] with complex arithmetic
                psB = pB.tile([NQ, 1024], F32, name="psB", tag="fB")
                fr = psB[:, 0:416]
                fi = psB[:, 512:928]
                fr_terms, fi_terms = [], []
                for k in range(2):
                    ur = u_sb[:, :, k, 0:52]
                    ui = u_sb[:, :, k, 52:104]
                    ck = wv_sb[:, k, 0:101]
                    sk = wv_sb[:, k, 101:202]
                    snk = wv_sb[:, k, 202:303]
                    fr_terms += [(ck, ur), (sk, ui)]
                    fi_terms += [(ck, ui), (snk, ur)]
                for i, (lhsT, rhs) in enumerate(fr_terms):
                    nc.tensor.matmul(fr, lhsT, rhs, start=(i == 0), stop=(i == 3))
                for i, (lhsT, rhs) in enumerate(fi_terms):
                    nc.tensor.matmul(fi, lhsT, rhs, start=(i == 0), stop=(i == 3))

                # |F|*MSCALE in fp16: one prescaled ACT square over the
                # (fr, fi) pair, fp16 add on DVE, sqrt on ACT
                m2 = wpool.tile([NQ, 2, 416], F16, name="m2", tag="m2")
                pair = psB.rearrange("p (a x) -> p a x", a=2)[:, :, 0:416]
                nc.scalar.activation(m2[:], pair, AF.Square, scale=MSCALE)
                nc.vector.tensor_add(m2[:, 0], m2[:, 0], m2[:, 1])
                mag = wpool.tile([NQ, 416], F16, name="mag", tag="mag")
                nc.scalar.sqrt(mag[:], m2[:, 0])

                # crop sum: both q-window matmuls accumulate in PSUM,
                # then one windowed free-dim reduce -> y row slice
                mag3 = mag.rearrange("p (c x) -> p c x", c=GS)
                g2 = pS.tile([1, 400], F32, name="g2", tag="G")
                nc.tensor.matmul(
                    g2[:], r1_sb[:, 0:1], mag3[:, :, 0:50], start=True, stop=False
                )
                nc.tensor.matmul(
                    g2[:], r1_sb[:, 1:2], mag3[:, :, 1:51], start=False, stop=True
                )
                ga = g2.rearrange("p (c x) -> p c x", c=GS)
                nc.vector.reduce_sum(
                    y_sb[0:1, GS * g:GS * (g + 1)], ga, axis=mybir.AxisListType.X
                )

            def emit_se(b):
                # SE block (mean divisor 1e4 and MSCALE folded into w1t)
                y_sb = ys[b]
                yT_ps = pS.tile([C, 1], F32, name="yT_ps", tag="se")
                nc.tensor.transpose(yT_ps[:], y_sb[:], id1_sb[:])
                y_col = wpool.tile([C, 1], F32, name="y_col", tag="se2")
                nc.vector.tensor_copy(y_col[:], yT_ps[:])
                h_ps = pS.tile([4, 1], F32, name="h_ps", tag="se")
                nc.tensor.matmul(h_ps[:], w1t_sb, y_col[:], start=True, stop=True)
                h_sb = wpool.tile([4, 1], F32, name="h_sb", tag="se3")
                # relu(h + b1) without an ACT Relu table:
                nc.vector.scalar_tensor_tensor(
                    h_sb[:], h_ps[:], b1c_sb, zeros4_sb[:], ALU.add, ALU.max
                )
                sarg_ps = pS.tile([1, C], F32, name="sarg_ps", tag="se")
                nc.tensor.matmul(sarg_ps[:], h_sb[:], w2t_sb, start=True, stop=True)
                sarg_sb = wpool.tile([1, C], F32, name="sarg_sb", tag="se4")
                nc.vector.tensor_add(sarg_sb[:], sarg_ps[:], b2r_sb)
                s_row = wpool.tile([1, C], F32, name="s_row", tag="se5")
                nc.scalar.activation(s_row[:], sarg_sb[:], AF.Sigmoid)
                sb_ps = pS.tile([128, C], F32, name="sb_ps", tag="se")
                nc.tensor.matmul(
                    sb_ps[:], ones128_sb[:], s_row[:], start=True, stop=True
                )
                s_b = wpool.tile([128, C], F32, name="s_b", tag="se6")
                nc.vector.tensor_copy(s_b[:], sb_ps[:])
                sbs[b] = s_b

            def emit_scale(b, g):
                # in-place fp16 per-channel scale: DVE TENSOR_SCALAR with
                # per-partition f32 AP scalar (fast path, ~340ns/channel)
                xr = xrs[(b, g)]
                s_b = sbs[b]
                for j in range(GS):
                    c = GS * g + j
                    nc.vector.tensor_scalar_mul(
                        xr[:, j], xr[:, j], s_b[:, c:c + 1]
                    )

            # software pipeline: A(u+1) before B/mag/crop(u); SE(0) as soon
            # as batch-0 crops are done; batch-0 scale interleaved with
            # batch-1's FFT stream.
            units = [(b, g) for b in range(BPC) for g in range(NG)]
            for u, (b, g) in enumerate(units):
                emit_A(b, g)
                if u >= 1:
                    emit_Bmagcrop(*units[u - 1])
                if u == NG:
                    emit_se(0)
                if NG + 1 <= u:
                    emit_scale(0, u - NG - 1)
            emit_Bmagcrop(*units[-1])
            emit_scale(0, NG - 1)
            emit_se(1)
            for g in range(NG):
                emit_scale(1, g)

            # output DMAs: sync HW queue, FIFO behind the 16 inputs;
            # each trigger waits only on its group's scale completion.
            for b in range(BPC):
                for g in range(NG):
                    nc.sync.dma_start(
                        out_d[b, :, GS * g:GS * (g + 1)], xrs[(b, g)][:]
                    )

    nc.compile()
    return nc


_NC = None


def _get_nc():
    global _NC
    if _NC is None:
        _NC = _build_nc()
    return _NC


def _execute(inputs, trace=False):
    x = np.asarray(inputs["x"], dtype=np.float32)
    consts = _build_consts(
        np.asarray(inputs["w1"]), np.asarray(inputs["b1"]),
        np.asarray(inputs["w2"]), np.asarray(inputs["b2"]),
    )
    in_maps = []
    for i in range(N_CORES):
        xs = x[BPC * i:BPC * (i + 1)]
        # [b, c, (p k), w] -> [b, p, c, k, w]  (h = 2p + k), fp16
        xs = np.ascontiguousarray(
            xs.reshape(BPC, C, 128, 2, W).transpose(0, 2, 1, 3, 4),
            dtype=np.float16,
        )
        m = {"x": xs}
        m.update(consts)
        in_maps.append(m)
    nc = _get_nc()
    res = run_bass_kernel_spmd(nc, in_maps, core_ids=list(range(N_CORES)), trace=trace)
    outs = []
    for i in range(N_CORES):
        o = res.results[i]["out"]  # [b, p, c, k, w] fp16
        o = o.transpose(0, 2, 1, 3, 4).reshape(BPC, C, H, W).astype(np.float32)
        outs.append(o)
    out = np.concatenate(outs, axis=0)
    return out, res


def kernel(x, w1, b1, w2, b2):
    out, _ = _execute({"x": x, "w1": w1, "b1": b1, "w2": w2, "b2": b2}, trace=False)
    return out
